# revision 40
# baseline (speedup 1.0000x reference)
"""Causal self-attention + depthwise-conv + out-proj fused TRN2 kernel.

Model (B=4, T=2048, C=1024, H=16, D=64, conv K=4):
    qkv = x @ W_qkv.T ; causal softmax attention per head ;
    y2 = attn + causal_depthwise_conv(attn) + conv_b ; out = y2 @ W_out.T

Sharding over 8 NeuronCores: core c -> (batch b = c//2, head-group g = c%2).
Each core computes q/k/v for its 8 heads (bf16 matmuls against x[b].T),
bf16 flash-style causal attention in transposed [d, t] layout (exp softmax
without max subtraction - logits are O(1)), the depthwise conv as fused
per-partition scalar multiply-adds on the Pool engine with the residual
folded into the lag-0 tap, then a pairwise AllGather of the 512-channel
activation and half of the output projection columns.

Layout notes:
  - scores are computed transposed: S^T[k, q] = K^T.T @ Q^T so that the AV
    matmul can consume exp(S^T) directly as the moving operand.
  - causal masking adds a {0, -30000} staircase onto the diagonal psum
    strips on the Pool engine before the exp.
  - the AV stationary is [V_h | ones]: rows 0-63 of the psum get attn^T,
    rows 64-127 get 64 replicas of the softmax denominator, so the
    normalization is a reciprocal + lane-wise multiply.
  - engine budget: PE does matmuls only; Act does the exps (+ wqk/outT DMA
    issue on its HWDGE queue); Pool does conv, masks and the collectives;
    DVE does psum drains, reciprocal and the normalize.
  - out-proj is split: the 6 psum-chain matmuls over pairs 0-2 run during
    pair-3's attention (partials parked in SBUF); only pair-3's two slabs
    remain after the final AllGather, shrinking the tail.
"""

import numpy as np
import ml_dtypes

import concourse.bacc as bacc
import concourse.mybir as mybir
import concourse.tile as tile
from concourse.bass_utils import run_bass_kernel_spmd

F32 = mybir.dt.float32
BF16 = mybir.dt.bfloat16
MULT = mybir.AluOpType.mult
ADD = mybir.AluOpType.add

B, T, C, H, D, K = 4, 2048, 1024, 16, 64, 4
HC = H // 2  # heads per core (8)
CC = C // 2  # channels per core (512)
NEG = -30000.0
NCORES = 8
REPLICA_GROUPS = [[0, 1], [2, 3], [4, 5], [6, 7]]
NTB = T // 512  # 512-wide t blocks (4)
NTT = T // 128  # 128-wide t tiles (16)
NCT = C // 128  # 128-wide input-channel tiles (8)
NPAIR = 4  # head pairs per core

_NC_CACHE = {}


def build(reps=1, qkv_dt=BF16, sim_collective=False):
    nc = bacc.Bacc(None, num_devices=NCORES)

    xT_d = nc.dram_tensor("xT", [C, T], qkv_dt, kind="ExternalInput")
    wqk_d = nc.dram_tensor("wqk", [C, 1024], qkv_dt, kind="ExternalInput")
    wv_d = nc.dram_tensor("wv", [C, CC], qkv_dt, kind="ExternalInput")
    wout_d = nc.dram_tensor("wout", [C, CC], BF16, kind="ExternalInput")
    masks_d = nc.dram_tensor("masks", [128, 512], BF16, kind="ExternalInput")
    masksp_d = nc.dram_tensor("masksp", [128, 512], BF16,
                              kind="ExternalInput")
    ident_d = nc.dram_tensor("ident", [128, 128], BF16, kind="ExternalInput")
    convw_d = nc.dram_tensor("convw", [128, NPAIR, K], F32, kind="ExternalInput")
    convb_d = nc.dram_tensor("convb", [128, NPAIR], F32, kind="ExternalInput")
    outT_d = nc.dram_tensor("outT", [CC, T], BF16, kind="ExternalOutput")

    with tile.TileContext(nc) as tc:
        with (
            tc.tile_pool(name="consts", bufs=1) as consts,
            tc.tile_pool(name="work", bufs=2) as work,
            tc.tile_pool(name="ps_st", bufs=2, space="PSUM") as ps_st,
            tc.tile_pool(name="ps_att", bufs=1, space="PSUM") as ps_att,
            tc.tile_pool(name="ps_mm", bufs=2, space="PSUM") as ps_mm,
            tc.tile_pool(name="dram", bufs=1, space="DRAM") as dram,
        ):
            # ---------- constants / big loads ----------
            # xT first on the SP HWDGE queue: per-ct DMAs so the first qk
            # chains can pace with the arrivals
            xT = consts.tile([128, NCT, T], xT_d.dtype, tag="xT")
            for ct in range(NCT):
                nc.sync.dma_start(xT[:, ct, :], xT_d[128 * ct : 128 * ct + 128, :])
            # small consts go on the sync queue BEHIND xT (all are first
            # needed after the first scores, ~1us after xT completes); the
            # scalar HWDGE queue stays free for the latency-critical wqk0
            masksp = consts.tile([128, 512], BF16, tag="masksp")
            nc.sync.dma_start(masksp[:], masksp_d[:])
            ident = consts.tile([128, 128], BF16, tag="ident")
            nc.sync.dma_start(ident[:], ident_d[:])
            masks = consts.tile([128, 512], BF16, tag="masks")
            nc.sync.dma_start(masks[:], masks_d[:])
            convw = consts.tile([128, NPAIR, K], F32, tag="convw")
            nc.sync.dma_start(convw[:], convw_d[:])
            convb = consts.tile([128, NPAIR], F32, tag="convb")
            nc.sync.dma_start(convb[:], convb_d[:])
            # hoist the Exp activation-table load into the DMA wait
            warm = consts.tile([1, 8], F32, tag="warm")
            nc.vector.memset(warm[:], 0.0)
            warm2 = consts.tile([1, 8], F32, tag="warm2")
            nc.scalar.activation(
                out=warm2[:], in_=warm[:],
                func=mybir.ActivationFunctionType.Exp, scale=1.0,
            )

            for rep in range(reps):
                _emit_body(nc, tc, consts, work, ps_st, ps_att, ps_mm, dram,
                           locals(), rep, sim_collective=sim_collective)

    nc.compile()
    return nc


def _emit_body(nc, tc, consts, work, ps_st, ps_att, ps_mm, dram, env, rep,
               sim_collective=False):
    xT = env["xT"]
    masks = env["masks"]
    masksp = env["masksp"]
    ident = env["ident"]
    convw = env["convw"]
    convb = env["convb"]
    wv_d = env["wv_d"]
    wqk_d = env["wqk_d"]
    wout_d = env["wout_d"]
    outT_d = env["outT_d"]
    R = f"r{rep}_"

    # wv shares its slot with wout (wv is dead once V is computed).
    # It rides the sync HWDGE ring BEHIND xT: the q/k chains (gated by xT)
    # get the full DMA bandwidth first; wv lands just in time for the
    # first V-projection consumed by pair-0 qb0's AV
    wv = consts.tile([128, NCT, CC], wv_d.dtype, tag="w2", name=R + "wv")
    for ct in range(NCT):
        nc.sync.dma_start(wv[:, ct, :], wv_d[128 * ct : 128 * ct + 128, :])

    # ---------- V projection emitted lazily (interleaved with pair-0
    # attention: qb only consumes v_ones[kt <= 4qb+3]) ----------
    v_ones = consts.tile([128, NTT, HC, 128], BF16, tag="v_ones",
                         name=R + "v_ones")
    nc.vector.memset(v_ones[:, :, :, 0:64], 1.0)

    def emit_vproj(tt):
        vps = ps_mm.tile([128, HC, 64], F32, tag="mm", name=f"{R}vps{tt}")
        for ct in range(NCT):
            nc.tensor.matmul(
                vps[:],
                xT[:, ct, tt * 128 : tt * 128 + 128],
                wv[:, ct, :],
                start=(ct == 0),
                stop=(ct == NCT - 1),
            )
        nc.vector.tensor_copy(v_ones[:, tt, :, 64:128], vps[:])

    # SBUF home for the allgathered conv activations of pairs 0-2; pair-3
    # quarters land in their own per-qb tiles so the out-proj finals carry
    # exact (non-coarsened) DMA dependencies
    y2all = consts.tile([128, NPAIR - 1, 2, T], BF16, tag="y2all",
                        name=R + "y2all")
    y2q3 = [
        consts.tile([128, 2, 512], BF16, tag=f"y2q3_{qb}",
                    name=f"{R}y2q3_{qb}")
        for qb in range(NTB - 1)
    ]
    # the last block's slabs arrive as two 256-col halves so the tail
    # pipeline (conv -> collective -> finals) runs at half granularity
    y2q3h = [
        consts.tile([128, 2, 256], BF16, tag=f"y2q3h{hh}",
                    name=f"{R}y2q3h{hh}")
        for hh in range(2)
    ]

    def fetch_wqk(p):
        # single DMA on the Act HWDGE queue (doesn't contend with xT/y2)
        wqk = work.tile([128, NCT, 256], wqk_d.dtype, tag="wqk", bufs=2,
                        name=f"{R}wqk{p}")
        nc.scalar.dma_start(
            wqk[:],
            wqk_d.rearrange("(n p) m -> p n m", p=128)[
                :, :, 256 * p : 256 * p + 256
            ],
        )
        return wqk

    def emit_qk_chain(p, wqk, qT, kT, fs, tb):
        dst = qT if fs == 0 else kT
        ps = ps_mm.tile([128, 512], F32, tag="mm", name=f"{R}qkps{p}_{fs}_{tb}")
        for ct in range(NCT):
            nc.tensor.matmul(
                ps[:],
                wqk[:, ct, 128 * fs : 128 * fs + 128],
                xT[:, ct, 512 * tb : 512 * tb + 512],
                start=(ct == 0),
                stop=(ct == NCT - 1),
            )
        nc.vector.tensor_copy(dst[:, 512 * tb : 512 * tb + 512], ps[:])

    def emit_qk_chain0_interleaved(wqk, qT, kT):
        # both tb=0 chains paced ct-by-ct with the xT DMA arrivals: each
        # chain's ct-k matmul runs as soon as xT ct k lands instead of the
        # fs=1 chain queueing behind the whole fs=0 chain
        pss = [
            ps_mm.tile([128, 512], F32, tag="mm", name=f"{R}qk0ps{fs}")
            for fs in range(2)
        ]
        for ct in range(NCT):
            for fs in range(2):
                nc.tensor.matmul(
                    pss[fs][:],
                    wqk[:, ct, 128 * fs : 128 * fs + 128],
                    xT[:, ct, 0:512],
                    start=(ct == 0),
                    stop=(ct == NCT - 1),
                )
        nc.vector.tensor_copy(qT[:, 0:512], pss[0][:])
        nc.vector.tensor_copy(kT[:, 0:512], pss[1][:])

    def make_qk(p):
        qT = work.tile([128, T], BF16, tag="qT", bufs=2, name=f"{R}qT{p}")
        kT = work.tile([128, T], BF16, tag="kT", bufs=2, name=f"{R}kT{p}")
        return qT, kT

    def emit_conv(p, tb, yt, y2dst, dst0):
        # causal depthwise conv + bias with the +1 residual folded into the
        # lag-0 tap, on Pool (SBUF-only engine: no scalar_tensor_tensor /
        # PSUM there, so tensor_scalar multiplies + tensor_add chain).
        # Small tap terms accumulate first in bf16 (they are ~0.02 scale);
        # the full-magnitude lag-0 term sees only the final rounding.
        t0 = 512 * tb
        ta = work.tile([128, 512], BF16, tag="cva", bufs=2,
                       name=f"{R}cva{p}_{tb}")
        tb_ = work.tile([128, 512], BF16, tag="cvb", bufs=2,
                        name=f"{R}cvb{p}_{tb}")
        y2sb = work.tile([128, 512], BF16, tag="y2sb", bufs=4,
                         name=f"{R}y2sb{p}_{tb}")
        lo = 3 if t0 == 0 else 0
        if lo:
            nc.gpsimd.memset(ta[:, 0:lo], 0.0)
        nc.gpsimd.tensor_scalar_mul(
            ta[:, lo:512], yt[:, t0 + lo - 3 : t0 + 509], convw[:, p, 3:4]
        )
        for lag in (2, 1):
            lo = lag if t0 == 0 else 0
            if lo:
                nc.gpsimd.memset(tb_[:, 0:lo], 0.0)
            nc.gpsimd.tensor_scalar_mul(
                tb_[:, lo:512],
                yt[:, t0 + lo - lag : t0 + 512 - lag],
                convw[:, p, lag : lag + 1],
            )
            nc.gpsimd.tensor_add(out=ta[:], in0=ta[:], in1=tb_[:])
        nc.gpsimd.tensor_scalar(
            out=tb_[:], in0=yt[:, t0 : t0 + 512],
            scalar1=convw[:, p, 0:1], scalar2=convb[:, p : p + 1],
            op0=MULT, op1=ADD,
        )
        nc.gpsimd.tensor_add(out=y2sb[:], in0=ta[:], in1=tb_[:])
        nc.sync.dma_start(y2dst[:, dst0 : dst0 + 512], y2sb[:])

    def emit_conv_dve(p, tb, yt, y2dst, dst0, t0=None, w=512):
        # DVE variant (fused scalar_tensor_tensor -> shortest serial
        # chain; it sits on the critical tail). t0/w select a sub-span
        # for the fine-grained final blocks.
        if t0 is None:
            t0 = 512 * tb
        acc = work.tile([128, 512], F32, tag="cvacc", bufs=2,
                        name=f"{R}cvd{p}_{tb}_{t0}")
        y2sb = work.tile([128, 512], BF16, tag="y2sb", bufs=4,
                         name=f"{R}y2sb{p}_{tb}_{t0}")
        nc.vector.tensor_scalar(
            out=acc[:, 0:w], in0=yt[:, t0 : t0 + w],
            scalar1=convw[:, p, 0:1], scalar2=convb[:, p : p + 1],
            op0=MULT, op1=ADD,
        )
        for lag in (1, 2):
            lo = max(0, lag - t0)
            nc.vector.scalar_tensor_tensor(
                out=acc[:, lo:w],
                in0=yt[:, t0 + lo - lag : t0 + w - lag],
                scalar=convw[:, p, lag : lag + 1],
                in1=acc[:, lo:w],
                op0=MULT, op1=ADD,
            )
        lo3 = max(0, 3 - t0)
        if lo3:
            nc.vector.tensor_copy(y2sb[:, 0:lo3], acc[:, 0:lo3])
        nc.vector.scalar_tensor_tensor(
            out=y2sb[:, lo3:w],
            in0=yt[:, t0 + lo3 - 3 : t0 + w - 3],
            scalar=convw[:, p, 3:4],
            in1=acc[:, lo3:w],
            op0=MULT, op1=ADD,
        )
        nc.sync.dma_start(y2dst[:, dst0 : dst0 + w], y2sb[:, 0:w])

    # out-proj partial accumulator over pairs 0-2
    out_acc = consts.tile([128, NTB, 4, 512], F32, tag="oacc",
                          name=R + "out_acc")

    def allgather(src, dst, tsz):
        if sim_collective:
            # stand-in for TimelineSim (single-core, no collectives):
            # same-size DRAM->DRAM traffic with the same dependencies
            nc.gpsimd.dma_start(dst[0:128, :], src[:, :])
            nc.gpsimd.dma_start(dst[128:256, :], src[:, :])
        else:
            nc.gpsimd.collective_compute(
                "AllGather",
                mybir.AluOpType.bypass,
                replica_groups=REPLICA_GROUPS,
                ins=[src.opt()],
                outs=[dst.opt()],
            )

    def emit_oproj_partial(wout, tb, ot):
        t0 = 512 * tb
        ops_ = ps_mm.tile([128, 512], F32, tag="mm", name=f"{R}opsA{tb}_{ot}")
        for gs in range(6):
            p_, par = gs // 2, gs % 2
            nc.tensor.matmul(
                ops_[:],
                wout[:, gs, 128 * ot : 128 * ot + 128],
                y2all[:, p_, par, t0 : t0 + 512],
                start=(gs == 0),
                stop=(gs == 5),
            )
        nc.vector.tensor_copy(out_acc[:, tb, ot, :], ops_[:])

    def emit_oproj_final_half(wout, ot, hh):
        t0 = 1536 + 256 * hh
        ops_ = ps_mm.tile([128, 256], F32, tag="mm",
                          name=f"{R}opsC{ot}_{hh}")
        for gs in (6, 7):
            nc.tensor.matmul(
                ops_[:],
                wout[:, gs, 128 * ot : 128 * ot + 128],
                y2q3h[hh][:, gs - 6, :],
                start=(gs == 6),
                stop=(gs == 7),
            )
        osb = work.tile([128, 256], BF16, tag="osbh", bufs=4,
                        name=f"{R}osbh{ot}_{hh}")
        nc.vector.tensor_add(osb[:], ops_[:],
                             out_acc[:, 3, ot, 256 * hh : 256 * hh + 256])
        nc.scalar.dma_start(
            outT_d[128 * ot : 128 * ot + 128, t0 : t0 + 256], osb[:]
        )

    def emit_oproj_final(wout, tb, ot):
        t0 = 512 * tb
        ops_ = ps_mm.tile([128, 512], F32, tag="mm", name=f"{R}opsB{tb}_{ot}")
        for gs in (6, 7):
            nc.tensor.matmul(
                ops_[:],
                wout[:, gs, 128 * ot : 128 * ot + 128],
                y2q3[tb][:, gs - 6, :],
                start=(gs == 6),
                stop=(gs == 7),
            )
        osb = work.tile([128, 512], BF16, tag="osb", bufs=4,
                        name=f"{R}osb{tb}_{ot}")
        nc.vector.tensor_add(osb[:], ops_[:], out_acc[:, tb, ot, :])
        nc.scalar.dma_start(
            outT_d[128 * ot : 128 * ot + 128, t0 : t0 + 512], osb[:]
        )

    # pairs 0-2: the qb3 conv + collective is deferred into the next
    # pair's qb0 so the boundary DVE queue (masks/recip of the new pair)
    # isn't stuck behind a conv that nothing needs for another ~30us
    defer = [None]

    # only pair 0 / tb 0 is projected eagerly; every other q/k chain is
    # emitted just-in-time inside the qb loops (chain (fs, tb=qb) is
    # exactly what attention qb consumes first)
    wqk_next = fetch_wqk(0)
    qk_next = make_qk(0)
    emit_qk_chain0_interleaved(wqk_next, *qk_next)
    wout = None
    for p in range(NPAIR):
        wqk, (qT, kT) = wqk_next, qk_next
        if p + 1 < NPAIR:
            wqk_next = fetch_wqk(p + 1)
            qk_next = make_qk(p + 1)
        if p == 1:
            # gathered-order W_out into the slot wv vacated after the
            # lazy V projection completed (end of pair 0)
            wout = consts.tile([128, NCT, CC], BF16, tag="w2", name=R + "wout")
            nc.sync.dma_start(
                wout[:], wout_d.rearrange("(n p) m -> p n m", p=128)
            )

        # ---------- attention for the two heads of pair p ----------
        yt = work.tile([128, T], BF16, tag="yt", bufs=2, name=f"{R}yt{p}")
        y2loc = None
        if p == 0:
            y2loc = dram.tile([128, T], BF16, tag=f"y2loc{p}",
                              name=f"{R}y2loc{p}")

        for qb in range(NTB):
            q0 = 512 * qb
            # filler PE units (complete psum chains) spread between the
            # attention grp units so the exp pipeline is never starved by
            # a burst of projection matmuls at the qb boundary
            fillers = []
            if qb < NTB - 1:
                # own q/k chains for the next time-block (one-qb lookahead)
                fillers += [
                    lambda fs=fs: emit_qk_chain(p, wqk, qT, kT, fs, qb + 1)
                    for fs in range(2)
                ]
            elif p + 1 < NPAIR:
                # next pair's tb0 chains, ready when its attention starts
                fillers += [
                    lambda fs=fs, w=wqk_next, qk=qk_next:
                        emit_qk_chain(p + 1, w, qk[0], qk[1], fs, 0)
                    for fs in range(2)
                ]
            if p == 3:
                # partials (tb) need y2all[p0/p1] (home early in pair 3) and
                # y2all[p2, half tb//2] (half collectives fire after p2 qb1 /
                # qb3); finals (tb) additionally need pair-3's own per-qb
                # collective for quarter tb
                part = lambda tb, ot: (
                    lambda: emit_oproj_partial(wout, tb, ot))
                fin = lambda tb, ot: (
                    lambda: emit_oproj_final(wout, tb, ot))
                if qb == 0:
                    fillers += [part(0, ot) for ot in range(4)]
                elif qb == 1:
                    fillers += [part(1, ot) for ot in range(4)]
                elif qb == 2:
                    fillers += [part(2, ot) for ot in range(4)]
                    fillers += [fin(0, ot) for ot in range(4)]
                    fillers += [fin(1, ot) for ot in range(4)]
                elif qb == 3:
                    fillers += [part(3, ot) for ot in range(4)]
                    # tb2 finals as late-qb3 fillers: emitted before qb3's
                    # own loads so their subtile deps stay exact
                    fillers += [fin(2, ot) for ot in range(4)]
            att = [
                ps_att.tile([128, 512], F32, tag=f"att{h}", bufs=1,
                            name=f"{R}att{p}_{qb}_{h}")
                for h in range(2)
            ]
            ngrp = 2 * qb + 2
            emitted = 0
            for grp in range(ngrp):
                if p == 0 and grp >= 2 * qb:
                    # V tiles consumed by this grp's AV matmuls
                    emit_vproj(2 * grp)
                    emit_vproj(2 * grp + 1)
                # both heads' score matmuls adjacent: 64-row stationaries
                # at base partitions 0/64 -> disjoint PE row groups, can
                # run concurrently in the array
                sts = []
                for h in range(2):
                    hp = 64 * h
                    st = ps_st.tile([128, 1024], F32, tag="st", bufs=2,
                                    name=f"{R}st{p}_{qb}_{grp}_{h}")
                    w0s = []
                    for half in range(2):
                        kt = 2 * grp + half
                        w0 = max(0, 128 * (kt - 4 * qb))
                        w0s.append(w0)
                        base = 512 * half
                        if qb == 0:
                            # pair/qb boundary: prefill the causal
                            # staircase (first 128 cols of masksp, zeros
                            # after) into the psum with an identity-
                            # stationary matmul so the exp never waits DVE
                            nc.tensor.matmul(
                                st[:, base + w0 : base + 512],
                                ident[:, :],
                                masksp[:, 0 : 512 - w0],
                                start=True,
                                stop=False,
                            )
                        nc.tensor.matmul(
                            st[:, base + w0 : base + 512],
                            kT[hp : hp + 64, 128 * kt : 128 * kt + 128],
                            qT[hp : hp + 64, q0 + w0 : q0 + 512],
                            start=(qb != 0),
                            stop=True,
                        )
                    sts.append((st, w0s))
                for h in range(2):
                    st, w0s = sts[h]
                    if grp >= 2 * qb and qb > 0:
                        # diagonal strip: staircase mask added on DVE
                        # (PSUM: Pool cannot access it)
                        b = grp - 2 * qb
                        for half in range(2):
                            i = 2 * b + half
                            c0 = 512 * half + w0s[half]
                            nc.vector.tensor_add(
                                out=st[:, c0 : c0 + 128],
                                in0=st[:, c0 : c0 + 128],
                                in1=masks[:, 128 * i : 128 * i + 128],
                            )
                    pt = work.tile([128, 1024], BF16, tag="pt", bufs=8,
                                   name=f"{R}pt{p}_{qb}_{grp}_{h}")
                    nc.scalar.activation(
                        out=pt[:, w0s[0] : 1024],
                        in_=st[:, w0s[0] : 1024],
                        func=mybir.ActivationFunctionType.Exp,
                        scale=0.125,
                    )
                    for half in range(2):
                        kt = 2 * grp + half
                        w0 = w0s[half]
                        base = 512 * half
                        nc.tensor.matmul(
                            att[h][:, w0:512],
                            v_ones[:, kt, 2 * p + h, :],
                            pt[:, base + w0 : base + 512],
                            start=(kt == 0),
                            stop=(kt == 4 * qb + 3),
                        )
                want = (grp + 1) * len(fillers) // ngrp
                while emitted < want:
                    fillers[emitted]()
                    emitted += 1
                if qb == 0 and grp == 0 and defer[0] is not None:
                    defer[0]()
                    defer[0] = None
            for h in range(2):
                rec = work.tile([64, 512], F32, tag="rec", bufs=2,
                                name=f"{R}rec{p}_{qb}_{h}")
                nc.vector.reciprocal_approx_fast(rec[:], att[h][0:64, :])
                nc.vector.tensor_mul(
                    out=yt[64 * h : 64 * h + 64, q0 : q0 + 512],
                    in0=att[h][64:128, :],
                    in1=rec[:],
                )

            # conv block qb inline for every pair: spreads Pool work evenly
            # and lets the collectives fire as early as their data is ready
            def conv_block(p=p, qb=qb, q0=q0, yt=yt, y2loc=y2loc):
                if p == NPAIR - 1 and qb == NTB - 1:
                    # final block at 256-col granularity: each half's
                    # conv -> store -> AllGather -> finals chain overlaps
                    # the other's
                    for hh in range(2):
                        t0 = q0 + 256 * hh
                        y2lq = dram.tile([128, 256], BF16,
                                         tag=f"y2locq3h{hh}",
                                         name=f"{R}y2locq3h{hh}")
                        emit_conv_dve(p, qb, yt, y2lq, 0, t0=t0, w=256)
                        y2g = dram.tile([256, 256], BF16,
                                        tag=f"y2gq3h{hh}",
                                        name=f"{R}y2gq3h{hh}")
                        allgather(y2lq, y2g, 256)
                        nc.sync.dma_start(
                            y2q3h[hh][:, :, :],
                            y2g.rearrange("(g p) t -> p g t", p=128),
                        )
                    return
                if p > 0:
                    # per-qb AllGather for pairs 1-3: each 512-block's slab
                    # is exchanged as soon as its conv completes, so the
                    # out-proj partials/finals scheduled into later qbs find
                    # their inputs home (collective input must be contiguous
                    # DRAM -> per-qb tile)
                    y2locq = dram.tile([128, 512], BF16,
                                       tag=f"y2locq{p}_{qb}",
                                       name=f"{R}y2locq{p}_{qb}")
                    emit_conv_dve(p, qb, yt, y2locq, 0)
                    y2g = dram.tile([256, 512], BF16, tag=f"y2gq{p}_{qb}",
                                    name=f"{R}y2gq{p}_{qb}")
                    allgather(y2locq, y2g, 512)
                    dst = (y2q3[qb][:, :, :] if p == NPAIR - 1
                           else y2all[:, p, :, q0 : q0 + 512])
                    nc.sync.dma_start(
                        dst, y2g.rearrange("(g p) t -> p g t", p=128)
                    )
                else:
                    emit_conv_dve(p, qb, yt, y2loc, q0)
                    if qb == NTB - 1:
                        # pair 0: single AllGather of the whole slab
                        y2g = dram.tile([256, T], BF16, tag="y2g0",
                                        name=f"{R}y2g0")
                        allgather(y2loc, y2g, T)
                        nc.sync.dma_start(
                            y2all[:, p, :, :],
                            y2g.rearrange("(g p) t -> p g t", p=128),
                        )

            if p < NPAIR - 1 and qb == NTB - 1:
                defer[0] = conv_block
            else:
                conv_block()

    # ---------- output projection tail: pair-3 slab for tb 3 ----------
    for hh in range(2):
        for ot in range(4):
            emit_oproj_final_half(wout, ot, hh)


def _make_masks():
    kp = np.arange(128)[:, None]
    col = np.arange(128)[None, :]
    masks = np.zeros((128, 512), np.float32)
    for i in range(4):
        masks[:, 128 * i : 128 * i + 128] = np.where(kp > col, NEG, 0.0)
    return masks.astype(ml_dtypes.bfloat16)


def _make_masksp():
    # prefill layout: staircase in cols 0:128, zeros after
    kp = np.arange(128)[:, None]
    col = np.arange(128)[None, :]
    m = np.zeros((128, 512), np.float32)
    m[:, 0:128] = np.where(kp > col, NEG, 0.0)
    return m.astype(ml_dtypes.bfloat16)


def prepare_in_maps(x, W_qkv, W_out, conv_w, conv_b, qkv_np=ml_dtypes.bfloat16):
    x = np.asarray(x, np.float32)
    W_qkv = np.asarray(W_qkv, np.float32)
    W_out = np.asarray(W_out, np.float32)
    conv_w = np.asarray(conv_w, np.float32).reshape(C, K)
    conv_b = np.asarray(conv_b, np.float32)

    masks = _make_masks()

    # gathered channel order: row r of y2g stack -> global channel
    perm = np.empty(C, np.int64)
    for r in range(C):
        p, parity, within = r // 256, (r % 256) // 128, r % 128
        perm[r] = 512 * parity + 128 * p + within

    in_maps = []
    for core in range(NCORES):
        b, g = core // 2, core % 2
        xT = np.ascontiguousarray(x[b].T)  # [C, T]
        # wqk: cols [256p:256p+128] = q rows of pair p (.T), then k rows
        wqk = np.empty((C, 1024), np.float32)
        for p in range(NPAIR):
            r0 = 64 * (8 * g + 2 * p)
            wqk[:, 256 * p : 256 * p + 128] = W_qkv[r0 : r0 + 128, :].T
            wqk[:, 256 * p + 128 : 256 * p + 256] = W_qkv[
                1024 + r0 : 1024 + r0 + 128, :
            ].T
        wv = np.ascontiguousarray(W_qkv[2048 + CC * g : 2048 + CC * g + CC, :].T)
        # W_out columns for this core's output slice, rows in gathered order
        wout = np.ascontiguousarray(
            W_out[CC * g : CC * g + CC, :].T[perm, :]
        ).astype(ml_dtypes.bfloat16)
        # conv taps per channel-partition, lag-indexed; +1 residual on lag 0
        convw = np.zeros((128, NPAIR, K), np.float32)
        convb2 = np.zeros((128, NPAIR), np.float32)
        for p in range(NPAIR):
            base = CC * g + 128 * p
            for lag in range(K):
                w = conv_w[base : base + 128, K - 1 - lag]
                if lag == 0:
                    w = w + 1.0
                convw[:, p, lag] = w
            convb2[:, p] = conv_b[base : base + 128]
        in_maps.append(
            {
                "xT": xT.astype(qkv_np),
                "wqk": wqk.astype(qkv_np),
                "wv": wv.astype(qkv_np),
                "wout": wout,
                "masks": masks,
                "masksp": _make_masksp(),
                "ident": np.eye(128, dtype=ml_dtypes.bfloat16),
                "convw": convw,
                "convb": convb2,
            }
        )
    return in_maps


def assemble_output(results):
    out = np.empty((B, T, C), np.float32)
    for core in range(NCORES):
        b, g = core // 2, core % 2
        outT = np.asarray(results[core]["outT"], np.float32)  # [CC, T]
        out[b, :, CC * g : CC * g + CC] = outT.T
    return out


def kernel(x, W_qkv, W_out, conv_w, conv_b):
    if "nc" not in _NC_CACHE:
        _NC_CACHE["nc"] = build()
    nc = _NC_CACHE["nc"]
    in_maps = prepare_in_maps(x, W_qkv, W_out, conv_w, conv_b)
    res = run_bass_kernel_spmd(nc, in_maps, list(range(NCORES)))
    return assemble_output(res.results)


# revision 41
# speedup vs baseline: 1.0198x; 1.0198x over previous
"""Causal self-attention + depthwise-conv + out-proj fused TRN2 kernel.

Model (B=4, T=2048, C=1024, H=16, D=64, conv K=4):
    qkv = x @ W_qkv.T ; causal softmax attention per head ;
    y2 = attn + causal_depthwise_conv(attn) + conv_b ; out = y2 @ W_out.T

Sharding over 8 NeuronCores: core c -> (batch b = c//2, head-group g = c%2).
Each core computes q/k/v for its 8 heads (bf16 matmuls against x[b].T),
bf16 flash-style causal attention in transposed [d, t] layout (exp softmax
without max subtraction - logits are O(1)), the depthwise conv as fused
per-partition scalar multiply-adds on DVE with the residual folded into
the lag-0 tap, then pairwise AllGathers of the 512-channel activation
(per 512-col block for pairs 1-3) and half of the out-proj columns.

Layout notes:
  - scores are computed transposed: S^T[k, q] = K^T.T @ Q^T so that the AV
    matmul can consume exp(S^T) directly as the moving operand.
  - causal masking adds a {0, -30000} staircase onto the diagonal psum
    strips (DVE), except at qb0 where the staircase is PREFILLED into the
    psum by an identity-stationary matmul so the boundary exp chain never
    waits on the DVE queue.
  - the AV stationary is [V_h | ones]: rows 0-63 of the psum get attn^T,
    rows 64-127 get 64 replicas of the softmax denominator, so the
    normalization is a reciprocal + lane-wise multiply.
  - engine budget: PE does matmuls (scores/AV/projections/mask prefill);
    Act does the exps (+ wqk/outT DMA issue on its HWDGE queue); Pool
    issues the collectives; DVE does psum drains, masks, reciprocal,
    normalize and the conv.
  - projection/out-proj matmul chains are emitted as "filler" units spread
    between attention grps so the exp pipeline is never starved by bursts;
    q/k chains run one time-block ahead of their consumer.
  - out-proj is split: the 6 psum-chain matmuls over pairs 0-2 run during
    pair-3's attention (partials parked in SBUF fp32); only pair-3's two
    slabs remain, and the last 512-block is processed as two 256-col
    halves so the tail conv->AllGather->finals chain is fine-grained.
"""

import numpy as np
import ml_dtypes

import concourse.bacc as bacc
import concourse.mybir as mybir
import concourse.tile as tile
from concourse.bass_utils import run_bass_kernel_spmd

F32 = mybir.dt.float32
BF16 = mybir.dt.bfloat16
MULT = mybir.AluOpType.mult
ADD = mybir.AluOpType.add

B, T, C, H, D, K = 4, 2048, 1024, 16, 64, 4
HC = H // 2  # heads per core (8)
CC = C // 2  # channels per core (512)
NEG = -30000.0
NCORES = 8
REPLICA_GROUPS = [[0, 1], [2, 3], [4, 5], [6, 7]]
NTB = T // 512  # 512-wide t blocks (4)
NTT = T // 128  # 128-wide t tiles (16)
NCT = C // 128  # 128-wide input-channel tiles (8)
NPAIR = 4  # head pairs per core

_NC_CACHE = {}


def build(reps=1, qkv_dt=BF16, sim_collective=False):
    nc = bacc.Bacc(None, num_devices=NCORES)

    xT_d = nc.dram_tensor("xT", [C, T], qkv_dt, kind="ExternalInput")
    wqk_d = nc.dram_tensor("wqk", [C, 1024], qkv_dt, kind="ExternalInput")
    wv_d = nc.dram_tensor("wv", [C, CC], qkv_dt, kind="ExternalInput")
    wout_d = nc.dram_tensor("wout", [C, CC], BF16, kind="ExternalInput")
    masks_d = nc.dram_tensor("masks", [128, 512], BF16, kind="ExternalInput")
    masksp_d = nc.dram_tensor("masksp", [128, 512], BF16,
                              kind="ExternalInput")
    ident_d = nc.dram_tensor("ident", [128, 128], BF16, kind="ExternalInput")
    convw_d = nc.dram_tensor("convw", [128, NPAIR, K], F32, kind="ExternalInput")
    convb_d = nc.dram_tensor("convb", [128, NPAIR], F32, kind="ExternalInput")
    outT_d = nc.dram_tensor("outT", [CC, T], BF16, kind="ExternalOutput")

    with tile.TileContext(nc) as tc:
        with (
            tc.tile_pool(name="consts", bufs=1) as consts,
            tc.tile_pool(name="work", bufs=2) as work,
            tc.tile_pool(name="ps_st", bufs=2, space="PSUM") as ps_st,
            tc.tile_pool(name="ps_att", bufs=1, space="PSUM") as ps_att,
            tc.tile_pool(name="ps_mm", bufs=2, space="PSUM") as ps_mm,
            tc.tile_pool(name="dram", bufs=1, space="DRAM") as dram,
        ):
            # ---------- constants / big loads ----------
            # xT first on the SP HWDGE queue: per-ct DMAs so the first qk
            # chains can pace with the arrivals
            xT = consts.tile([128, NCT, T], xT_d.dtype, tag="xT")
            for ct in range(NCT):
                nc.sync.dma_start(xT[:, ct, :], xT_d[128 * ct : 128 * ct + 128, :])
            # small consts go on the sync queue BEHIND xT (all are first
            # needed after the first scores, ~1us after xT completes); the
            # scalar HWDGE queue stays free for the latency-critical wqk0
            masksp = consts.tile([128, 512], BF16, tag="masksp")
            nc.sync.dma_start(masksp[:], masksp_d[:])
            ident = consts.tile([128, 128], BF16, tag="ident")
            nc.sync.dma_start(ident[:], ident_d[:])
            masks = consts.tile([128, 512], BF16, tag="masks")
            nc.sync.dma_start(masks[:], masks_d[:])
            convw = consts.tile([128, NPAIR, K], F32, tag="convw")
            nc.sync.dma_start(convw[:], convw_d[:])
            convb = consts.tile([128, NPAIR], F32, tag="convb")
            nc.sync.dma_start(convb[:], convb_d[:])
            # hoist the Exp activation-table load into the DMA wait
            warm = consts.tile([1, 8], F32, tag="warm")
            nc.vector.memset(warm[:], 0.0)
            warm2 = consts.tile([1, 8], F32, tag="warm2")
            nc.scalar.activation(
                out=warm2[:], in_=warm[:],
                func=mybir.ActivationFunctionType.Exp, scale=1.0,
            )

            for rep in range(reps):
                _emit_body(nc, tc, consts, work, ps_st, ps_att, ps_mm, dram,
                           locals(), rep, sim_collective=sim_collective)

    nc.compile()
    return nc


def _emit_body(nc, tc, consts, work, ps_st, ps_att, ps_mm, dram, env, rep,
               sim_collective=False):
    xT = env["xT"]
    masks = env["masks"]
    masksp = env["masksp"]
    ident = env["ident"]
    convw = env["convw"]
    convb = env["convb"]
    wv_d = env["wv_d"]
    wqk_d = env["wqk_d"]
    wout_d = env["wout_d"]
    outT_d = env["outT_d"]
    R = f"r{rep}_"

    # wv shares its slot with wout (wv is dead once V is computed).
    # It rides the sync HWDGE ring BEHIND xT: the q/k chains (gated by xT)
    # get the full DMA bandwidth first; wv lands just in time for the
    # first V-projection consumed by pair-0 qb0's AV
    wv = consts.tile([128, NCT, CC], wv_d.dtype, tag="w2", name=R + "wv")
    for ct in range(NCT):
        nc.sync.dma_start(wv[:, ct, :], wv_d[128 * ct : 128 * ct + 128, :])

    # ---------- V projection emitted lazily (interleaved with pair-0
    # attention: qb only consumes v_ones[kt <= 4qb+3]) ----------
    v_ones = consts.tile([128, NTT, HC, 128], BF16, tag="v_ones",
                         name=R + "v_ones")
    nc.vector.memset(v_ones[:, :, :, 0:64], 1.0)

    def emit_vproj(tt):
        vps = ps_mm.tile([128, HC, 64], F32, tag="mm", name=f"{R}vps{tt}")
        for ct in range(NCT):
            nc.tensor.matmul(
                vps[:],
                xT[:, ct, tt * 128 : tt * 128 + 128],
                wv[:, ct, :],
                start=(ct == 0),
                stop=(ct == NCT - 1),
            )
        nc.vector.tensor_copy(v_ones[:, tt, :, 64:128], vps[:])

    # SBUF home for the allgathered conv activations of pairs 0-2; pair-3
    # quarters land in their own per-qb tiles so the out-proj finals carry
    # exact (non-coarsened) DMA dependencies
    y2all = consts.tile([128, NPAIR - 1, 2, T], BF16, tag="y2all",
                        name=R + "y2all")
    y2q3 = [
        consts.tile([128, 2, 512], BF16, tag=f"y2q3_{qb}",
                    name=f"{R}y2q3_{qb}")
        for qb in range(NTB - 1)
    ]
    # the last block's slabs arrive as two 256-col halves so the tail
    # pipeline (conv -> collective -> finals) runs at half granularity
    y2q3h = [
        consts.tile([128, 2, 256], BF16, tag=f"y2q3h{hh}",
                    name=f"{R}y2q3h{hh}")
        for hh in range(2)
    ]

    def fetch_wqk(p):
        # single DMA on the Act HWDGE queue (doesn't contend with xT/y2)
        wqk = work.tile([128, NCT, 256], wqk_d.dtype, tag="wqk", bufs=2,
                        name=f"{R}wqk{p}")
        nc.scalar.dma_start(
            wqk[:],
            wqk_d.rearrange("(n p) m -> p n m", p=128)[
                :, :, 256 * p : 256 * p + 256
            ],
        )
        return wqk

    def emit_qk_chain(p, wqk, qT, kT, fs, tb):
        dst = qT if fs == 0 else kT
        ps = ps_mm.tile([128, 512], F32, tag="mm", name=f"{R}qkps{p}_{fs}_{tb}")
        for ct in range(NCT):
            nc.tensor.matmul(
                ps[:],
                wqk[:, ct, 128 * fs : 128 * fs + 128],
                xT[:, ct, 512 * tb : 512 * tb + 512],
                start=(ct == 0),
                stop=(ct == NCT - 1),
            )
        nc.vector.tensor_copy(dst[:, 512 * tb : 512 * tb + 512], ps[:])

    def emit_qk_chain0_interleaved(wqk, qT, kT):
        # both tb=0 chains paced ct-by-ct with the xT DMA arrivals: each
        # chain's ct-k matmul runs as soon as xT ct k lands instead of the
        # fs=1 chain queueing behind the whole fs=0 chain
        pss = [
            ps_mm.tile([128, 512], F32, tag="mm", name=f"{R}qk0ps{fs}")
            for fs in range(2)
        ]
        for ct in range(NCT):
            for fs in range(2):
                nc.tensor.matmul(
                    pss[fs][:],
                    wqk[:, ct, 128 * fs : 128 * fs + 128],
                    xT[:, ct, 0:512],
                    start=(ct == 0),
                    stop=(ct == NCT - 1),
                )
        nc.vector.tensor_copy(qT[:, 0:512], pss[0][:])
        nc.vector.tensor_copy(kT[:, 0:512], pss[1][:])

    def make_qk(p):
        qT = work.tile([128, T], BF16, tag="qT", bufs=2, name=f"{R}qT{p}")
        kT = work.tile([128, T], BF16, tag="kT", bufs=2, name=f"{R}kT{p}")
        return qT, kT

    def emit_conv(p, tb, yt, y2dst, dst0):
        # causal depthwise conv + bias with the +1 residual folded into the
        # lag-0 tap, on Pool (SBUF-only engine: no scalar_tensor_tensor /
        # PSUM there, so tensor_scalar multiplies + tensor_add chain).
        # Small tap terms accumulate first in bf16 (they are ~0.02 scale);
        # the full-magnitude lag-0 term sees only the final rounding.
        t0 = 512 * tb
        ta = work.tile([128, 512], BF16, tag="cva", bufs=2,
                       name=f"{R}cva{p}_{tb}")
        tb_ = work.tile([128, 512], BF16, tag="cvb", bufs=2,
                        name=f"{R}cvb{p}_{tb}")
        y2sb = work.tile([128, 512], BF16, tag="y2sb", bufs=4,
                         name=f"{R}y2sb{p}_{tb}")
        lo = 3 if t0 == 0 else 0
        if lo:
            nc.gpsimd.memset(ta[:, 0:lo], 0.0)
        nc.gpsimd.tensor_scalar_mul(
            ta[:, lo:512], yt[:, t0 + lo - 3 : t0 + 509], convw[:, p, 3:4]
        )
        for lag in (2, 1):
            lo = lag if t0 == 0 else 0
            if lo:
                nc.gpsimd.memset(tb_[:, 0:lo], 0.0)
            nc.gpsimd.tensor_scalar_mul(
                tb_[:, lo:512],
                yt[:, t0 + lo - lag : t0 + 512 - lag],
                convw[:, p, lag : lag + 1],
            )
            nc.gpsimd.tensor_add(out=ta[:], in0=ta[:], in1=tb_[:])
        nc.gpsimd.tensor_scalar(
            out=tb_[:], in0=yt[:, t0 : t0 + 512],
            scalar1=convw[:, p, 0:1], scalar2=convb[:, p : p + 1],
            op0=MULT, op1=ADD,
        )
        nc.gpsimd.tensor_add(out=y2sb[:], in0=ta[:], in1=tb_[:])
        nc.sync.dma_start(y2dst[:, dst0 : dst0 + 512], y2sb[:])

    def emit_conv_dve(p, tb, yt, y2dst, dst0, t0=None, w=512):
        # DVE variant (fused scalar_tensor_tensor -> shortest serial
        # chain; it sits on the critical tail). t0/w select a sub-span
        # for the fine-grained final blocks.
        if t0 is None:
            t0 = 512 * tb
        acc = work.tile([128, 512], F32, tag="cvacc", bufs=2,
                        name=f"{R}cvd{p}_{tb}_{t0}")
        y2sb = work.tile([128, 512], BF16, tag="y2sb", bufs=4,
                         name=f"{R}y2sb{p}_{tb}_{t0}")
        nc.vector.tensor_scalar(
            out=acc[:, 0:w], in0=yt[:, t0 : t0 + w],
            scalar1=convw[:, p, 0:1], scalar2=convb[:, p : p + 1],
            op0=MULT, op1=ADD,
        )
        for lag in (1, 2):
            lo = max(0, lag - t0)
            nc.vector.scalar_tensor_tensor(
                out=acc[:, lo:w],
                in0=yt[:, t0 + lo - lag : t0 + w - lag],
                scalar=convw[:, p, lag : lag + 1],
                in1=acc[:, lo:w],
                op0=MULT, op1=ADD,
            )
        lo3 = max(0, 3 - t0)
        if lo3:
            nc.vector.tensor_copy(y2sb[:, 0:lo3], acc[:, 0:lo3])
        nc.vector.scalar_tensor_tensor(
            out=y2sb[:, lo3:w],
            in0=yt[:, t0 + lo3 - 3 : t0 + w - 3],
            scalar=convw[:, p, 3:4],
            in1=acc[:, lo3:w],
            op0=MULT, op1=ADD,
        )
        nc.sync.dma_start(y2dst[:, dst0 : dst0 + w], y2sb[:, 0:w])

    # out-proj partial accumulator over pairs 0-2
    out_acc = consts.tile([128, NTB, 4, 512], F32, tag="oacc",
                          name=R + "out_acc")

    def allgather(src, dst, tsz):
        if sim_collective:
            # stand-in for TimelineSim (single-core, no collectives):
            # same-size DRAM->DRAM traffic with the same dependencies
            nc.gpsimd.dma_start(dst[0:128, :], src[:, :])
            nc.gpsimd.dma_start(dst[128:256, :], src[:, :])
        else:
            nc.gpsimd.collective_compute(
                "AllGather",
                mybir.AluOpType.bypass,
                replica_groups=REPLICA_GROUPS,
                ins=[src.opt()],
                outs=[dst.opt()],
            )

    def emit_oproj_partial(wout, tb, ot):
        t0 = 512 * tb
        ops_ = ps_mm.tile([128, 512], F32, tag="mm", name=f"{R}opsA{tb}_{ot}")
        for gs in range(6):
            p_, par = gs // 2, gs % 2
            nc.tensor.matmul(
                ops_[:],
                wout[:, gs, 128 * ot : 128 * ot + 128],
                y2all[:, p_, par, t0 : t0 + 512],
                start=(gs == 0),
                stop=(gs == 5),
            )
        nc.vector.tensor_copy(out_acc[:, tb, ot, :], ops_[:])

    def emit_oproj_final_half(wout, ot, hh):
        t0 = 1536 + 256 * hh
        ops_ = ps_mm.tile([128, 256], F32, tag="mm",
                          name=f"{R}opsC{ot}_{hh}")
        for gs in (6, 7):
            nc.tensor.matmul(
                ops_[:],
                wout[:, gs, 128 * ot : 128 * ot + 128],
                y2q3h[hh][:, gs - 6, :],
                start=(gs == 6),
                stop=(gs == 7),
            )
        osb = work.tile([128, 256], BF16, tag="osbh", bufs=4,
                        name=f"{R}osbh{ot}_{hh}")
        nc.vector.tensor_add(osb[:], ops_[:],
                             out_acc[:, 3, ot, 256 * hh : 256 * hh + 256])
        nc.scalar.dma_start(
            outT_d[128 * ot : 128 * ot + 128, t0 : t0 + 256], osb[:]
        )

    def emit_oproj_final(wout, tb, ot):
        t0 = 512 * tb
        ops_ = ps_mm.tile([128, 512], F32, tag="mm", name=f"{R}opsB{tb}_{ot}")
        for gs in (6, 7):
            nc.tensor.matmul(
                ops_[:],
                wout[:, gs, 128 * ot : 128 * ot + 128],
                y2q3[tb][:, gs - 6, :],
                start=(gs == 6),
                stop=(gs == 7),
            )
        osb = work.tile([128, 512], BF16, tag="osb", bufs=4,
                        name=f"{R}osb{tb}_{ot}")
        nc.vector.tensor_add(osb[:], ops_[:], out_acc[:, tb, ot, :])
        nc.scalar.dma_start(
            outT_d[128 * ot : 128 * ot + 128, t0 : t0 + 512], osb[:]
        )

    # pairs 0-2: the qb3 conv + collective is deferred into the next
    # pair's qb0 so the boundary DVE queue (masks/recip of the new pair)
    # isn't stuck behind a conv that nothing needs for another ~30us
    defer = [None]

    # only pair 0 / tb 0 is projected eagerly; every other q/k chain is
    # emitted just-in-time inside the qb loops (chain (fs, tb=qb) is
    # exactly what attention qb consumes first)
    wqk_next = fetch_wqk(0)
    qk_next = make_qk(0)
    emit_qk_chain0_interleaved(wqk_next, *qk_next)
    wout = None
    for p in range(NPAIR):
        wqk, (qT, kT) = wqk_next, qk_next
        if p + 1 < NPAIR:
            wqk_next = fetch_wqk(p + 1)
            qk_next = make_qk(p + 1)
        if p == 1:
            # gathered-order W_out into the slot wv vacated after the
            # lazy V projection completed (end of pair 0)
            wout = consts.tile([128, NCT, CC], BF16, tag="w2", name=R + "wout")
            nc.sync.dma_start(
                wout[:], wout_d.rearrange("(n p) m -> p n m", p=128)
            )

        # ---------- attention for the two heads of pair p ----------
        yt = work.tile([128, T], BF16, tag="yt", bufs=2, name=f"{R}yt{p}")
        y2loc = None
        if p == 0:
            y2loc = dram.tile([128, T], BF16, tag=f"y2loc{p}",
                              name=f"{R}y2loc{p}")

        for qb in range(NTB):
            q0 = 512 * qb
            # filler PE units (complete psum chains) spread between the
            # attention grp units so the exp pipeline is never starved by
            # a burst of projection matmuls at the qb boundary
            fillers = []
            if qb < NTB - 1:
                # own q/k chains for the next time-block (one-qb lookahead)
                fillers += [
                    lambda fs=fs: emit_qk_chain(p, wqk, qT, kT, fs, qb + 1)
                    for fs in range(2)
                ]
            elif p + 1 < NPAIR:
                # next pair's tb0 chains, ready when its attention starts
                fillers += [
                    lambda fs=fs, w=wqk_next, qk=qk_next:
                        emit_qk_chain(p + 1, w, qk[0], qk[1], fs, 0)
                    for fs in range(2)
                ]
            if p == 3:
                # partials (tb) need y2all[p0/p1] (home early in pair 3) and
                # y2all[p2, half tb//2] (half collectives fire after p2 qb1 /
                # qb3); finals (tb) additionally need pair-3's own per-qb
                # collective for quarter tb
                part = lambda tb, ot: (
                    lambda: emit_oproj_partial(wout, tb, ot))
                fin = lambda tb, ot: (
                    lambda: emit_oproj_final(wout, tb, ot))
                if qb == 0:
                    fillers += [part(0, ot) for ot in range(4)]
                elif qb == 1:
                    fillers += [part(1, ot) for ot in range(4)]
                elif qb == 2:
                    fillers += [part(2, ot) for ot in range(4)]
                    fillers += [fin(0, ot) for ot in range(4)]
                    fillers += [fin(1, ot) for ot in range(4)]
                elif qb == 3:
                    fillers += [part(3, ot) for ot in range(4)]
                    # tb2 finals as late-qb3 fillers: emitted before qb3's
                    # own loads so their subtile deps stay exact
                    fillers += [fin(2, ot) for ot in range(4)]
            att = [
                ps_att.tile([128, 512], F32, tag=f"att{h}", bufs=1,
                            name=f"{R}att{p}_{qb}_{h}")
                for h in range(2)
            ]
            ngrp = 2 * qb + 2
            emitted = 0
            for grp in range(ngrp):
                if p == 0 and grp >= 2 * qb:
                    # V tiles consumed by this grp's AV matmuls
                    emit_vproj(2 * grp)
                    emit_vproj(2 * grp + 1)
                # both heads' score matmuls adjacent: 64-row stationaries
                # at base partitions 0/64 -> disjoint PE row groups, can
                # run concurrently in the array
                sts = []
                for h in range(2):
                    hp = 64 * h
                    st = ps_st.tile([128, 1024], F32, tag="st", bufs=2,
                                    name=f"{R}st{p}_{qb}_{grp}_{h}")
                    w0s = []
                    for half in range(2):
                        kt = 2 * grp + half
                        w0 = max(0, 128 * (kt - 4 * qb))
                        w0s.append(w0)
                        base = 512 * half
                        if qb == 0:
                            # pair/qb boundary: prefill the causal
                            # staircase (first 128 cols of masksp, zeros
                            # after) into the psum with an identity-
                            # stationary matmul so the exp never waits DVE
                            nc.tensor.matmul(
                                st[:, base + w0 : base + 512],
                                ident[:, :],
                                masksp[:, 0 : 512 - w0],
                                start=True,
                                stop=False,
                            )
                        nc.tensor.matmul(
                            st[:, base + w0 : base + 512],
                            kT[hp : hp + 64, 128 * kt : 128 * kt + 128],
                            qT[hp : hp + 64, q0 + w0 : q0 + 512],
                            start=(qb != 0),
                            stop=True,
                        )
                    sts.append((st, w0s))
                for h in range(2):
                    st, w0s = sts[h]
                    if grp >= 2 * qb and qb > 0:
                        # diagonal strip: staircase mask added on DVE
                        # (PSUM: Pool cannot access it)
                        b = grp - 2 * qb
                        for half in range(2):
                            i = 2 * b + half
                            c0 = 512 * half + w0s[half]
                            nc.vector.tensor_add(
                                out=st[:, c0 : c0 + 128],
                                in0=st[:, c0 : c0 + 128],
                                in1=masks[:, 128 * i : 128 * i + 128],
                            )
                    pt = work.tile([128, 1024], BF16, tag="pt", bufs=8,
                                   name=f"{R}pt{p}_{qb}_{grp}_{h}")
                    if grp >= 2 * qb and w0s[0] >= 256:
                        # diagonal grp with a wide stale zone between the
                        # halves: two exps skip it (saves Act columns)
                        for half in range(2):
                            c0 = 512 * half + w0s[half]
                            nc.scalar.activation(
                                out=pt[:, c0 : 512 * half + 512],
                                in_=st[:, c0 : 512 * half + 512],
                                func=mybir.ActivationFunctionType.Exp,
                                scale=0.125,
                            )
                    else:
                        nc.scalar.activation(
                            out=pt[:, w0s[0] : 1024],
                            in_=st[:, w0s[0] : 1024],
                            func=mybir.ActivationFunctionType.Exp,
                            scale=0.125,
                        )
                    for half in range(2):
                        kt = 2 * grp + half
                        w0 = w0s[half]
                        base = 512 * half
                        nc.tensor.matmul(
                            att[h][:, w0:512],
                            v_ones[:, kt, 2 * p + h, :],
                            pt[:, base + w0 : base + 512],
                            start=(kt == 0),
                            stop=(kt == 4 * qb + 3),
                        )
                want = (grp + 1) * len(fillers) // ngrp
                while emitted < want:
                    fillers[emitted]()
                    emitted += 1
                if qb == 0 and grp == 0 and defer[0] is not None:
                    defer[0]()
                    defer[0] = None
            for h in range(2):
                rec = work.tile([64, 512], F32, tag="rec", bufs=2,
                                name=f"{R}rec{p}_{qb}_{h}")
                nc.vector.reciprocal_approx_fast(rec[:], att[h][0:64, :])
                nc.vector.tensor_mul(
                    out=yt[64 * h : 64 * h + 64, q0 : q0 + 512],
                    in0=att[h][64:128, :],
                    in1=rec[:],
                )

            # conv block qb inline for every pair: spreads Pool work evenly
            # and lets the collectives fire as early as their data is ready
            def conv_block(p=p, qb=qb, q0=q0, yt=yt, y2loc=y2loc):
                if p == NPAIR - 1 and qb == NTB - 1:
                    # final block at 256-col granularity: each half's
                    # conv -> store -> AllGather -> finals chain overlaps
                    # the other's
                    for hh in range(2):
                        t0 = q0 + 256 * hh
                        y2lq = dram.tile([128, 256], BF16,
                                         tag=f"y2locq3h{hh}",
                                         name=f"{R}y2locq3h{hh}")
                        emit_conv_dve(p, qb, yt, y2lq, 0, t0=t0, w=256)
                        y2g = dram.tile([256, 256], BF16,
                                        tag=f"y2gq3h{hh}",
                                        name=f"{R}y2gq3h{hh}")
                        allgather(y2lq, y2g, 256)
                        nc.sync.dma_start(
                            y2q3h[hh][:, :, :],
                            y2g.rearrange("(g p) t -> p g t", p=128),
                        )
                    return
                if p > 0:
                    # per-qb AllGather for pairs 1-3: each 512-block's slab
                    # is exchanged as soon as its conv completes, so the
                    # out-proj partials/finals scheduled into later qbs find
                    # their inputs home (collective input must be contiguous
                    # DRAM -> per-qb tile)
                    y2locq = dram.tile([128, 512], BF16,
                                       tag=f"y2locq{p}_{qb}",
                                       name=f"{R}y2locq{p}_{qb}")
                    emit_conv_dve(p, qb, yt, y2locq, 0)
                    y2g = dram.tile([256, 512], BF16, tag=f"y2gq{p}_{qb}",
                                    name=f"{R}y2gq{p}_{qb}")
                    allgather(y2locq, y2g, 512)
                    dst = (y2q3[qb][:, :, :] if p == NPAIR - 1
                           else y2all[:, p, :, q0 : q0 + 512])
                    nc.sync.dma_start(
                        dst, y2g.rearrange("(g p) t -> p g t", p=128)
                    )
                else:
                    emit_conv_dve(p, qb, yt, y2loc, q0)
                    if qb == NTB - 1:
                        # pair 0: single AllGather of the whole slab
                        y2g = dram.tile([256, T], BF16, tag="y2g0",
                                        name=f"{R}y2g0")
                        allgather(y2loc, y2g, T)
                        nc.sync.dma_start(
                            y2all[:, p, :, :],
                            y2g.rearrange("(g p) t -> p g t", p=128),
                        )

            if p < NPAIR - 1 and qb == NTB - 1:
                defer[0] = conv_block
            else:
                conv_block()

    # ---------- output projection tail: pair-3 slab for tb 3 ----------
    for hh in range(2):
        for ot in range(4):
            emit_oproj_final_half(wout, ot, hh)


def _make_masks():
    kp = np.arange(128)[:, None]
    col = np.arange(128)[None, :]
    masks = np.zeros((128, 512), np.float32)
    for i in range(4):
        masks[:, 128 * i : 128 * i + 128] = np.where(kp > col, NEG, 0.0)
    return masks.astype(ml_dtypes.bfloat16)


def _make_masksp():
    # prefill layout: staircase in cols 0:128, zeros after
    kp = np.arange(128)[:, None]
    col = np.arange(128)[None, :]
    m = np.zeros((128, 512), np.float32)
    m[:, 0:128] = np.where(kp > col, NEG, 0.0)
    return m.astype(ml_dtypes.bfloat16)


def prepare_in_maps(x, W_qkv, W_out, conv_w, conv_b, qkv_np=ml_dtypes.bfloat16):
    x = np.asarray(x, np.float32)
    W_qkv = np.asarray(W_qkv, np.float32)
    W_out = np.asarray(W_out, np.float32)
    conv_w = np.asarray(conv_w, np.float32).reshape(C, K)
    conv_b = np.asarray(conv_b, np.float32)

    masks = _make_masks()

    # gathered channel order: row r of y2g stack -> global channel
    perm = np.empty(C, np.int64)
    for r in range(C):
        p, parity, within = r // 256, (r % 256) // 128, r % 128
        perm[r] = 512 * parity + 128 * p + within

    in_maps = []
    for core in range(NCORES):
        b, g = core // 2, core % 2
        xT = np.ascontiguousarray(x[b].T)  # [C, T]
        # wqk: cols [256p:256p+128] = q rows of pair p (.T), then k rows
        wqk = np.empty((C, 1024), np.float32)
        for p in range(NPAIR):
            r0 = 64 * (8 * g + 2 * p)
            wqk[:, 256 * p : 256 * p + 128] = W_qkv[r0 : r0 + 128, :].T
            wqk[:, 256 * p + 128 : 256 * p + 256] = W_qkv[
                1024 + r0 : 1024 + r0 + 128, :
            ].T
        wv = np.ascontiguousarray(W_qkv[2048 + CC * g : 2048 + CC * g + CC, :].T)
        # W_out columns for this core's output slice, rows in gathered order
        wout = np.ascontiguousarray(
            W_out[CC * g : CC * g + CC, :].T[perm, :]
        ).astype(ml_dtypes.bfloat16)
        # conv taps per channel-partition, lag-indexed; +1 residual on lag 0
        convw = np.zeros((128, NPAIR, K), np.float32)
        convb2 = np.zeros((128, NPAIR), np.float32)
        for p in range(NPAIR):
            base = CC * g + 128 * p
            for lag in range(K):
                w = conv_w[base : base + 128, K - 1 - lag]
                if lag == 0:
                    w = w + 1.0
                convw[:, p, lag] = w
            convb2[:, p] = conv_b[base : base + 128]
        in_maps.append(
            {
                "xT": xT.astype(qkv_np),
                "wqk": wqk.astype(qkv_np),
                "wv": wv.astype(qkv_np),
                "wout": wout,
                "masks": masks,
                "masksp": _make_masksp(),
                "ident": np.eye(128, dtype=ml_dtypes.bfloat16),
                "convw": convw,
                "convb": convb2,
            }
        )
    return in_maps


def assemble_output(results):
    out = np.empty((B, T, C), np.float32)
    for core in range(NCORES):
        b, g = core // 2, core % 2
        outT = np.asarray(results[core]["outT"], np.float32)  # [CC, T]
        out[b, :, CC * g : CC * g + CC] = outT.T
    return out


def kernel(x, W_qkv, W_out, conv_w, conv_b):
    if "nc" not in _NC_CACHE:
        _NC_CACHE["nc"] = build()
    nc = _NC_CACHE["nc"]
    in_maps = prepare_in_maps(x, W_qkv, W_out, conv_w, conv_b)
    res = run_bass_kernel_spmd(nc, in_maps, list(range(NCORES)))
    return assemble_output(res.results)


# revision 45
# speedup vs baseline: 1.0807x; 1.0597x over previous
"""Causal self-attention + depthwise-conv + out-proj fused TRN2 kernel.

Model (B=4, T=2048, C=1024, H=16, D=64, conv K=4):
    qkv = x @ W_qkv.T ; causal softmax attention per head ;
    y2 = attn + causal_depthwise_conv(attn) + conv_b ; out = y2 @ W_out.T

Sharding over 8 NeuronCores: core c -> (batch b = c//2, head-group g = c%2).
Each core computes q/k/v for its 8 heads (bf16 matmuls against x[b].T),
bf16 flash-style causal attention in transposed [d, t] layout (exp softmax
without max subtraction - logits are O(1)), the depthwise conv as fused
per-partition scalar multiply-adds on DVE with the residual folded into
the lag-0 tap, then pairwise AllGathers of the 512-channel activation
(per 512-col block for pairs 1-3) and half of the out-proj columns.

Layout notes:
  - scores are computed transposed: S^T[k, q] = K^T.T @ Q^T so that the AV
    matmul can consume exp(S^T) directly as the moving operand.
  - causal masking adds a {0, -30000} staircase onto the diagonal psum
    strips (DVE), except at qb0 where the staircase is PREFILLED into the
    psum by an identity-stationary matmul so the boundary exp chain never
    waits on the DVE queue.
  - the AV stationary is [V_h | ones]: rows 0-63 of the psum get attn^T,
    rows 64-127 get 64 replicas of the softmax denominator, so the
    normalization is a reciprocal + lane-wise multiply.
  - engine budget: PE does matmuls (scores/AV/projections/mask prefill);
    Act does the exps (+ wqk/outT DMA issue on its HWDGE queue); Pool
    issues the collectives; DVE does psum drains, masks, reciprocal,
    normalize and the conv.
  - projection/out-proj matmul chains are emitted as "filler" units spread
    between attention grps so the exp pipeline is never starved by bursts;
    q/k chains run one time-block ahead of their consumer.
  - out-proj is split: the 6 psum-chain matmuls over pairs 0-2 run during
    pair-3's attention (partials parked in SBUF fp32); only pair-3's two
    slabs remain, and the last 512-block is processed as two 256-col
    halves so the tail conv->AllGather->finals chain is fine-grained.
"""

import numpy as np
import ml_dtypes

import concourse.bacc as bacc
import concourse.mybir as mybir
import concourse.tile as tile
from concourse.bass_utils import run_bass_kernel_spmd

F32 = mybir.dt.float32
BF16 = mybir.dt.bfloat16
MULT = mybir.AluOpType.mult
ADD = mybir.AluOpType.add

B, T, C, H, D, K = 4, 2048, 1024, 16, 64, 4
HC = H // 2  # heads per core (8)
CC = C // 2  # channels per core (512)
NEG = -30000.0
NCORES = 8
REPLICA_GROUPS = [[0, 1], [2, 3], [4, 5], [6, 7]]
NTB = T // 512  # 512-wide t blocks (4)
NTT = T // 128  # 128-wide t tiles (16)
NCT = C // 128  # 128-wide input-channel tiles (8)
NPAIR = 4  # head pairs per core

_NC_CACHE = {}


def build(reps=1, qkv_dt=BF16, sim_collective=False):
    nc = bacc.Bacc(None, num_devices=NCORES)

    xT_d = nc.dram_tensor("xT", [C, T], qkv_dt, kind="ExternalInput")
    wqk_d = nc.dram_tensor("wqk", [C, 1024], qkv_dt, kind="ExternalInput")
    wv_d = nc.dram_tensor("wv", [C, CC], qkv_dt, kind="ExternalInput")
    wout_d = nc.dram_tensor("wout", [C, CC], BF16, kind="ExternalInput")
    masks_d = nc.dram_tensor("masks", [128, 512], BF16, kind="ExternalInput")
    masksp_d = nc.dram_tensor("masksp", [128, 512], BF16,
                              kind="ExternalInput")
    ident_d = nc.dram_tensor("ident", [128, 128], BF16, kind="ExternalInput")
    convw_d = nc.dram_tensor("convw", [128, NPAIR, K], F32, kind="ExternalInput")
    convb_d = nc.dram_tensor("convb", [128, NPAIR], F32, kind="ExternalInput")
    outT_d = nc.dram_tensor("outT", [CC, T], BF16, kind="ExternalOutput")

    with tile.TileContext(nc) as tc:
        with (
            tc.tile_pool(name="consts", bufs=1) as consts,
            tc.tile_pool(name="work", bufs=2) as work,
            tc.tile_pool(name="ps_st", bufs=2, space="PSUM") as ps_st,
            tc.tile_pool(name="ps_att", bufs=1, space="PSUM") as ps_att,
            tc.tile_pool(name="ps_mm", bufs=2, space="PSUM") as ps_mm,
            tc.tile_pool(name="dram", bufs=1, space="DRAM") as dram,
        ):
            # ---------- constants / big loads ----------
            # xT first on the SP HWDGE queue: per-ct DMAs so the first qk
            # chains can pace with the arrivals
            xT = consts.tile([128, NCT, T], xT_d.dtype, tag="xT")
            for ct in range(NCT):
                nc.sync.dma_start(xT[:, ct, :], xT_d[128 * ct : 128 * ct + 128, :])
            # small consts go on the sync queue BEHIND xT (all are first
            # needed after the first scores, ~1us after xT completes); the
            # scalar HWDGE queue stays free for the latency-critical wqk0
            masksp = consts.tile([128, 512], BF16, tag="masksp")
            nc.sync.dma_start(masksp[:], masksp_d[:])
            ident = consts.tile([128, 128], BF16, tag="ident")
            nc.sync.dma_start(ident[:], ident_d[:])
            masks = consts.tile([128, 512], BF16, tag="masks")
            nc.sync.dma_start(masks[:], masks_d[:])
            convw = consts.tile([128, NPAIR, K], F32, tag="convw")
            nc.sync.dma_start(convw[:], convw_d[:])
            convb = consts.tile([128, NPAIR], F32, tag="convb")
            nc.sync.dma_start(convb[:], convb_d[:])
            # AV stationary home [V_h | ones]: allocated once; the ones
            # half is constant across reps, V halves are overwritten by
            # each rep's V-projection
            v_ones = consts.tile([128, NTT, HC, 128], BF16, tag="v_ones")
            nc.vector.memset(v_ones[:, :, :, 0:64], 1.0)
            # hoist the Exp activation-table load into the DMA wait
            warm = consts.tile([1, 8], F32, tag="warm")
            nc.vector.memset(warm[:], 0.0)
            warm2 = consts.tile([1, 8], F32, tag="warm2")
            nc.scalar.activation(
                out=warm2[:], in_=warm[:],
                func=mybir.ActivationFunctionType.Exp, scale=1.0,
            )

            for rep in range(reps):
                _emit_body(nc, tc, consts, work, ps_st, ps_att, ps_mm, dram,
                           locals(), rep, sim_collective=sim_collective)

    nc.compile()
    return nc


def _emit_body(nc, tc, consts, work, ps_st, ps_att, ps_mm, dram, env, rep,
               sim_collective=False):
    xT = env["xT"]
    masks = env["masks"]
    masksp = env["masksp"]
    ident = env["ident"]
    convw = env["convw"]
    convb = env["convb"]
    wv_d = env["wv_d"]
    wqk_d = env["wqk_d"]
    wout_d = env["wout_d"]
    outT_d = env["outT_d"]
    R = f"r{rep}_"

    # wv shares its slot with wout (wv is dead once V is computed).
    # It rides the sync HWDGE ring BEHIND xT: the q/k chains (gated by xT)
    # get the full DMA bandwidth first; wv lands just in time for the
    # first V-projection consumed by pair-0 qb0's AV
    wv = consts.tile([128, NCT, CC], wv_d.dtype, tag="w2", name=R + "wv")
    for ct in range(NCT):
        nc.sync.dma_start(wv[:, ct, :], wv_d[128 * ct : 128 * ct + 128, :])

    v_ones = env["v_ones"]

    # ---------- V projection emitted lazily (interleaved with pair-0
    # attention: qb only consumes v_ones[kt <= 4qb+3]) ----------
    def emit_vproj(tt):
        vps = ps_mm.tile([128, HC, 64], F32, tag="mm", name=f"{R}vps{tt}")
        for ct in range(NCT):
            nc.tensor.matmul(
                vps[:],
                xT[:, ct, tt * 128 : tt * 128 + 128],
                wv[:, ct, :],
                start=(ct == 0),
                stop=(ct == NCT - 1),
            )
        nc.vector.tensor_copy(v_ones[:, tt, :, 64:128], vps[:])

    # SBUF home for the allgathered conv activations of pairs 0-2; pair-3
    # quarters land in their own per-qb tiles so the out-proj finals carry
    # exact (non-coarsened) DMA dependencies
    y2all = consts.tile([128, NPAIR - 1, 2, T], BF16, tag="y2all",
                        name=R + "y2all")
    y2q3 = [
        consts.tile([128, 2, 512], BF16, tag=f"y2q3_{qb}",
                    name=f"{R}y2q3_{qb}")
        for qb in range(NTB - 1)
    ]
    # the last block's slabs arrive as two 256-col halves so the tail
    # pipeline (conv -> collective -> finals) runs at half granularity
    y2q3h = [
        consts.tile([128, 2, 256], BF16, tag=f"y2q3h{hh}",
                    name=f"{R}y2q3h{hh}")
        for hh in range(2)
    ]

    def fetch_wqk(p):
        # single DMA on the Act HWDGE queue (doesn't contend with xT/y2)
        wqk = work.tile([128, NCT, 256], wqk_d.dtype, tag="wqk", bufs=2,
                        name=f"{R}wqk{p}")
        nc.scalar.dma_start(
            wqk[:],
            wqk_d.rearrange("(n p) m -> p n m", p=128)[
                :, :, 256 * p : 256 * p + 256
            ],
        )
        return wqk

    def emit_qk_chain(p, wqk, qT, kT, fs, tb):
        dst = qT if fs == 0 else kT
        ps = ps_mm.tile([128, 512], F32, tag="mm", name=f"{R}qkps{p}_{fs}_{tb}")
        for ct in range(NCT):
            nc.tensor.matmul(
                ps[:],
                wqk[:, ct, 128 * fs : 128 * fs + 128],
                xT[:, ct, 512 * tb : 512 * tb + 512],
                start=(ct == 0),
                stop=(ct == NCT - 1),
            )
        nc.vector.tensor_copy(dst[:, 512 * tb : 512 * tb + 512], ps[:])

    def emit_qk_chain0_interleaved(wqk, qT, kT):
        # both tb=0 chains paced ct-by-ct with the xT DMA arrivals: each
        # chain's ct-k matmul runs as soon as xT ct k lands instead of the
        # fs=1 chain queueing behind the whole fs=0 chain
        pss = [
            ps_mm.tile([128, 512], F32, tag="mm", name=f"{R}qk0ps{fs}")
            for fs in range(2)
        ]
        for ct in range(NCT):
            for fs in range(2):
                nc.tensor.matmul(
                    pss[fs][:],
                    wqk[:, ct, 128 * fs : 128 * fs + 128],
                    xT[:, ct, 0:512],
                    start=(ct == 0),
                    stop=(ct == NCT - 1),
                )
        nc.vector.tensor_copy(qT[:, 0:512], pss[0][:])
        nc.vector.tensor_copy(kT[:, 0:512], pss[1][:])

    def make_qk(p):
        qT = work.tile([128, T], BF16, tag="qT", bufs=2, name=f"{R}qT{p}")
        kT = work.tile([128, T], BF16, tag="kT", bufs=2, name=f"{R}kT{p}")
        return qT, kT

    def emit_conv(p, tb, yt, y2dst, dst0):
        # causal depthwise conv + bias with the +1 residual folded into the
        # lag-0 tap, on Pool (SBUF-only engine: no scalar_tensor_tensor /
        # PSUM there, so tensor_scalar multiplies + tensor_add chain).
        # Small tap terms accumulate first in bf16 (they are ~0.02 scale);
        # the full-magnitude lag-0 term sees only the final rounding.
        t0 = 512 * tb
        ta = work.tile([128, 512], BF16, tag="cva", bufs=2,
                       name=f"{R}cva{p}_{tb}")
        tb_ = work.tile([128, 512], BF16, tag="cvb", bufs=2,
                        name=f"{R}cvb{p}_{tb}")
        y2sb = work.tile([128, 512], BF16, tag="y2sb", bufs=4,
                         name=f"{R}y2sb{p}_{tb}")
        lo = 3 if t0 == 0 else 0
        if lo:
            nc.gpsimd.memset(ta[:, 0:lo], 0.0)
        nc.gpsimd.tensor_scalar_mul(
            ta[:, lo:512], yt[:, t0 + lo - 3 : t0 + 509], convw[:, p, 3:4]
        )
        for lag in (2, 1):
            lo = lag if t0 == 0 else 0
            if lo:
                nc.gpsimd.memset(tb_[:, 0:lo], 0.0)
            nc.gpsimd.tensor_scalar_mul(
                tb_[:, lo:512],
                yt[:, t0 + lo - lag : t0 + 512 - lag],
                convw[:, p, lag : lag + 1],
            )
            nc.gpsimd.tensor_add(out=ta[:], in0=ta[:], in1=tb_[:])
        nc.gpsimd.tensor_scalar(
            out=tb_[:], in0=yt[:, t0 : t0 + 512],
            scalar1=convw[:, p, 0:1], scalar2=convb[:, p : p + 1],
            op0=MULT, op1=ADD,
        )
        nc.gpsimd.tensor_add(out=y2sb[:], in0=ta[:], in1=tb_[:])
        nc.sync.dma_start(y2dst[:, dst0 : dst0 + 512], y2sb[:])

    def emit_conv_dve(p, tb, yt, y2dst, dst0, t0=None, w=512):
        # DVE variant (fused scalar_tensor_tensor -> shortest serial
        # chain; it sits on the critical tail). t0/w select a sub-span
        # for the fine-grained final blocks.
        if t0 is None:
            t0 = 512 * tb
        acc = work.tile([128, 512], F32, tag="cvacc", bufs=2,
                        name=f"{R}cvd{p}_{tb}_{t0}")
        y2sb = work.tile([128, 512], BF16, tag="y2sb", bufs=4,
                         name=f"{R}y2sb{p}_{tb}_{t0}")
        nc.vector.tensor_scalar(
            out=acc[:, 0:w], in0=yt[:, t0 : t0 + w],
            scalar1=convw[:, p, 0:1], scalar2=convb[:, p : p + 1],
            op0=MULT, op1=ADD,
        )
        for lag in (1, 2):
            lo = max(0, lag - t0)
            nc.vector.scalar_tensor_tensor(
                out=acc[:, lo:w],
                in0=yt[:, t0 + lo - lag : t0 + w - lag],
                scalar=convw[:, p, lag : lag + 1],
                in1=acc[:, lo:w],
                op0=MULT, op1=ADD,
            )
        lo3 = max(0, 3 - t0)
        if lo3:
            nc.vector.tensor_copy(y2sb[:, 0:lo3], acc[:, 0:lo3])
        nc.vector.scalar_tensor_tensor(
            out=y2sb[:, lo3:w],
            in0=yt[:, t0 + lo3 - 3 : t0 + w - 3],
            scalar=convw[:, p, 3:4],
            in1=acc[:, lo3:w],
            op0=MULT, op1=ADD,
        )
        nc.sync.dma_start(y2dst[:, dst0 : dst0 + w], y2sb[:, 0:w])

    # out-proj partial accumulator over pairs 0-2
    out_acc = consts.tile([128, NTB, 4, 512], F32, tag="oacc",
                          name=R + "out_acc")

    def allgather(src, dst, tsz):
        if sim_collective:
            # stand-in for TimelineSim (single-core, no collectives):
            # same-size DRAM->DRAM traffic with the same dependencies
            nc.gpsimd.dma_start(dst[0:128, :], src[:, :])
            nc.gpsimd.dma_start(dst[128:256, :], src[:, :])
        else:
            nc.gpsimd.collective_compute(
                "AllGather",
                mybir.AluOpType.bypass,
                replica_groups=REPLICA_GROUPS,
                ins=[src.opt()],
                outs=[dst.opt()],
            )

    def emit_oproj_partial(wout, tb, ot):
        t0 = 512 * tb
        ops_ = ps_mm.tile([128, 512], F32, tag="mm", name=f"{R}opsA{tb}_{ot}")
        for gs in range(6):
            p_, par = gs // 2, gs % 2
            nc.tensor.matmul(
                ops_[:],
                wout[:, gs, 128 * ot : 128 * ot + 128],
                y2all[:, p_, par, t0 : t0 + 512],
                start=(gs == 0),
                stop=(gs == 5),
            )
        nc.vector.tensor_copy(out_acc[:, tb, ot, :], ops_[:])

    def emit_oproj_final_half(wout, ot, hh):
        t0 = 1536 + 256 * hh
        ops_ = ps_mm.tile([128, 256], F32, tag="mm",
                          name=f"{R}opsC{ot}_{hh}")
        for gs in (6, 7):
            nc.tensor.matmul(
                ops_[:],
                wout[:, gs, 128 * ot : 128 * ot + 128],
                y2q3h[hh][:, gs - 6, :],
                start=(gs == 6),
                stop=(gs == 7),
            )
        osb = work.tile([128, 256], BF16, tag="osbh", bufs=4,
                        name=f"{R}osbh{ot}_{hh}")
        nc.vector.tensor_add(osb[:], ops_[:],
                             out_acc[:, 3, ot, 256 * hh : 256 * hh + 256])
        nc.scalar.dma_start(
            outT_d[128 * ot : 128 * ot + 128, t0 : t0 + 256], osb[:]
        )

    def emit_oproj_final(wout, tb, ot):
        t0 = 512 * tb
        ops_ = ps_mm.tile([128, 512], F32, tag="mm", name=f"{R}opsB{tb}_{ot}")
        for gs in (6, 7):
            nc.tensor.matmul(
                ops_[:],
                wout[:, gs, 128 * ot : 128 * ot + 128],
                y2q3[tb][:, gs - 6, :],
                start=(gs == 6),
                stop=(gs == 7),
            )
        osb = work.tile([128, 512], BF16, tag="osb", bufs=4,
                        name=f"{R}osb{tb}_{ot}")
        nc.vector.tensor_add(osb[:], ops_[:], out_acc[:, tb, ot, :])
        nc.scalar.dma_start(
            outT_d[128 * ot : 128 * ot + 128, t0 : t0 + 512], osb[:]
        )

    # pairs 0-2: the qb3 conv + collective is deferred into the next
    # pair's qb0 so the boundary DVE queue (masks/recip of the new pair)
    # isn't stuck behind a conv that nothing needs for another ~30us
    defer = [None]

    # only pair 0 / tb 0 is projected eagerly; every other q/k chain is
    # emitted just-in-time inside the qb loops (chain (fs, tb=qb) is
    # exactly what attention qb consumes first)
    wqk_next = fetch_wqk(0)
    qk_next = make_qk(0)
    emit_qk_chain0_interleaved(wqk_next, *qk_next)
    wout = None
    for p in range(NPAIR):
        wqk, (qT, kT) = wqk_next, qk_next
        if p + 1 < NPAIR:
            wqk_next = fetch_wqk(p + 1)
            qk_next = make_qk(p + 1)
        if p == 1:
            # gathered-order W_out into the slot wv vacated after the
            # lazy V projection completed (end of pair 0)
            wout = consts.tile([128, NCT, CC], BF16, tag="w2", name=R + "wout")
            nc.sync.dma_start(
                wout[:], wout_d.rearrange("(n p) m -> p n m", p=128)
            )

        # ---------- attention for the two heads of pair p ----------
        yt = work.tile([128, T], BF16, tag="yt", bufs=2, name=f"{R}yt{p}")
        y2loc = None
        if p == 0:
            y2loc = dram.tile([128, T], BF16, tag=f"y2loc{p}",
                              name=f"{R}y2loc{p}")

        for qb in range(NTB):
            q0 = 512 * qb
            # filler PE units (complete psum chains) spread between the
            # attention grp units so the exp pipeline is never starved by
            # a burst of projection matmuls at the qb boundary
            fillers = []
            if qb < NTB - 1:
                # own q/k chains for the next time-block (one-qb lookahead)
                fillers += [
                    lambda fs=fs: emit_qk_chain(p, wqk, qT, kT, fs, qb + 1)
                    for fs in range(2)
                ]
            elif p + 1 < NPAIR:
                # next pair's tb0 chains, ready when its attention starts
                fillers += [
                    lambda fs=fs, w=wqk_next, qk=qk_next:
                        emit_qk_chain(p + 1, w, qk[0], qk[1], fs, 0)
                    for fs in range(2)
                ]
            if p == 3:
                # partials (tb) need y2all[p0/p1] (home early in pair 3) and
                # y2all[p2, half tb//2] (half collectives fire after p2 qb1 /
                # qb3); finals (tb) additionally need pair-3's own per-qb
                # collective for quarter tb
                part = lambda tb, ot: (
                    lambda: emit_oproj_partial(wout, tb, ot))
                fin = lambda tb, ot: (
                    lambda: emit_oproj_final(wout, tb, ot))
                if qb == 0:
                    fillers += [part(0, ot) for ot in range(4)]
                elif qb == 1:
                    fillers += [part(1, ot) for ot in range(4)]
                elif qb == 2:
                    fillers += [part(2, ot) for ot in range(4)]
                    fillers += [fin(0, ot) for ot in range(4)]
                    fillers += [fin(1, ot) for ot in range(4)]
                elif qb == 3:
                    fillers += [part(3, ot) for ot in range(4)]
                    # tb2 finals as late-qb3 fillers: emitted before qb3's
                    # own loads so their subtile deps stay exact
                    fillers += [fin(2, ot) for ot in range(4)]
            att = [
                ps_att.tile([128, 512], F32, tag=f"att{h}", bufs=1,
                            name=f"{R}att{p}_{qb}_{h}")
                for h in range(2)
            ]
            ngrp = 2 * qb + 2
            emitted = 0
            for grp in range(ngrp):
                if p == 0 and grp >= 2 * qb:
                    # V tiles consumed by this grp's AV matmuls
                    emit_vproj(2 * grp)
                    emit_vproj(2 * grp + 1)
                # both heads' score matmuls adjacent: 64-row stationaries
                # at base partitions 0/64 -> disjoint PE row groups, can
                # run concurrently in the array
                sts = []
                for h in range(2):
                    hp = 64 * h
                    st = ps_st.tile([128, 1024], F32, tag="st", bufs=2,
                                    name=f"{R}st{p}_{qb}_{grp}_{h}")
                    w0s = []
                    for half in range(2):
                        kt = 2 * grp + half
                        w0 = max(0, 128 * (kt - 4 * qb))
                        w0s.append(w0)
                        base = 512 * half
                        if qb == 0:
                            # pair/qb boundary: prefill the causal
                            # staircase (first 128 cols of masksp, zeros
                            # after) into the psum with an identity-
                            # stationary matmul so the exp never waits DVE
                            nc.tensor.matmul(
                                st[:, base + w0 : base + 512],
                                ident[:, :],
                                masksp[:, 0 : 512 - w0],
                                start=True,
                                stop=False,
                            )
                        nc.tensor.matmul(
                            st[:, base + w0 : base + 512],
                            kT[hp : hp + 64, 128 * kt : 128 * kt + 128],
                            qT[hp : hp + 64, q0 + w0 : q0 + 512],
                            start=(qb != 0),
                            stop=True,
                        )
                    sts.append((st, w0s))
                for h in range(2):
                    st, w0s = sts[h]
                    if grp >= 2 * qb and qb > 0:
                        # diagonal strip: staircase mask added on DVE
                        # (PSUM: Pool cannot access it)
                        b = grp - 2 * qb
                        for half in range(2):
                            i = 2 * b + half
                            c0 = 512 * half + w0s[half]
                            nc.vector.tensor_add(
                                out=st[:, c0 : c0 + 128],
                                in0=st[:, c0 : c0 + 128],
                                in1=masks[:, 128 * i : 128 * i + 128],
                            )
                    pt = work.tile([128, 1024], BF16, tag="pt", bufs=8,
                                   name=f"{R}pt{p}_{qb}_{grp}_{h}")
                    if grp >= 2 * qb and w0s[0] >= 256:
                        # diagonal grp with a wide stale zone between the
                        # halves: two exps skip it (saves Act columns)
                        for half in range(2):
                            c0 = 512 * half + w0s[half]
                            nc.scalar.activation(
                                out=pt[:, c0 : 512 * half + 512],
                                in_=st[:, c0 : 512 * half + 512],
                                func=mybir.ActivationFunctionType.Exp,
                                scale=0.125,
                            )
                    else:
                        nc.scalar.activation(
                            out=pt[:, w0s[0] : 1024],
                            in_=st[:, w0s[0] : 1024],
                            func=mybir.ActivationFunctionType.Exp,
                            scale=0.125,
                        )
                    for half in range(2):
                        kt = 2 * grp + half
                        w0 = w0s[half]
                        base = 512 * half
                        nc.tensor.matmul(
                            att[h][:, w0:512],
                            v_ones[:, kt, 2 * p + h, :],
                            pt[:, base + w0 : base + 512],
                            start=(kt == 0),
                            stop=(kt == 4 * qb + 3),
                        )
                want = (grp + 1) * len(fillers) // ngrp
                while emitted < want:
                    fillers[emitted]()
                    emitted += 1
                if qb == 0 and grp == 0 and defer[0] is not None:
                    defer[0]()
                    defer[0] = None
            for h in range(2):
                rec = work.tile([64, 512], F32, tag="rec", bufs=2,
                                name=f"{R}rec{p}_{qb}_{h}")
                nc.vector.reciprocal_approx_fast(rec[:], att[h][0:64, :])
                nc.vector.tensor_mul(
                    out=yt[64 * h : 64 * h + 64, q0 : q0 + 512],
                    in0=att[h][64:128, :],
                    in1=rec[:],
                )

            # conv block qb inline for every pair: spreads Pool work evenly
            # and lets the collectives fire as early as their data is ready
            def conv_block(p=p, qb=qb, q0=q0, yt=yt, y2loc=y2loc):
                if p == NPAIR - 1 and qb == NTB - 1:
                    # final block at 256-col granularity: each half's
                    # conv -> store -> AllGather -> finals chain overlaps
                    # the other's
                    for hh in range(2):
                        t0 = q0 + 256 * hh
                        y2lq = dram.tile([128, 256], BF16,
                                         tag=f"y2locq3h{hh}",
                                         name=f"{R}y2locq3h{hh}")
                        emit_conv_dve(p, qb, yt, y2lq, 0, t0=t0, w=256)
                        y2g = dram.tile([256, 256], BF16,
                                        tag=f"y2gq3h{hh}",
                                        name=f"{R}y2gq3h{hh}")
                        allgather(y2lq, y2g, 256)
                        nc.sync.dma_start(
                            y2q3h[hh][:, :, :],
                            y2g.rearrange("(g p) t -> p g t", p=128),
                        )
                    return
                if p > 0:
                    # per-qb AllGather for pairs 1-3: each 512-block's slab
                    # is exchanged as soon as its conv completes, so the
                    # out-proj partials/finals scheduled into later qbs find
                    # their inputs home (collective input must be contiguous
                    # DRAM -> per-qb tile)
                    y2locq = dram.tile([128, 512], BF16,
                                       tag=f"y2locq{p}_{qb}",
                                       name=f"{R}y2locq{p}_{qb}")
                    emit_conv_dve(p, qb, yt, y2locq, 0)
                    y2g = dram.tile([256, 512], BF16, tag=f"y2gq{p}_{qb}",
                                    name=f"{R}y2gq{p}_{qb}")
                    allgather(y2locq, y2g, 512)
                    dst = (y2q3[qb][:, :, :] if p == NPAIR - 1
                           else y2all[:, p, :, q0 : q0 + 512])
                    nc.sync.dma_start(
                        dst, y2g.rearrange("(g p) t -> p g t", p=128)
                    )
                else:
                    emit_conv_dve(p, qb, yt, y2loc, q0)
                    if qb == NTB - 1:
                        # pair 0: single AllGather of the whole slab
                        y2g = dram.tile([256, T], BF16, tag="y2g0",
                                        name=f"{R}y2g0")
                        allgather(y2loc, y2g, T)
                        nc.sync.dma_start(
                            y2all[:, p, :, :],
                            y2g.rearrange("(g p) t -> p g t", p=128),
                        )

            if p < NPAIR - 1 and qb == NTB - 1:
                defer[0] = conv_block
            else:
                conv_block()

    # ---------- output projection tail: pair-3 slab for tb 3 ----------
    for hh in range(2):
        for ot in range(4):
            emit_oproj_final_half(wout, ot, hh)


def _make_masks():
    kp = np.arange(128)[:, None]
    col = np.arange(128)[None, :]
    masks = np.zeros((128, 512), np.float32)
    for i in range(4):
        masks[:, 128 * i : 128 * i + 128] = np.where(kp > col, NEG, 0.0)
    return masks.astype(ml_dtypes.bfloat16)


def _make_masksp():
    # prefill layout: staircase in cols 0:128, zeros after
    kp = np.arange(128)[:, None]
    col = np.arange(128)[None, :]
    m = np.zeros((128, 512), np.float32)
    m[:, 0:128] = np.where(kp > col, NEG, 0.0)
    return m.astype(ml_dtypes.bfloat16)


def prepare_in_maps(x, W_qkv, W_out, conv_w, conv_b, qkv_np=ml_dtypes.bfloat16):
    x = np.asarray(x, np.float32)
    W_qkv = np.asarray(W_qkv, np.float32)
    W_out = np.asarray(W_out, np.float32)
    conv_w = np.asarray(conv_w, np.float32).reshape(C, K)
    conv_b = np.asarray(conv_b, np.float32)

    masks = _make_masks()

    # gathered channel order: row r of y2g stack -> global channel
    perm = np.empty(C, np.int64)
    for r in range(C):
        p, parity, within = r // 256, (r % 256) // 128, r % 128
        perm[r] = 512 * parity + 128 * p + within

    in_maps = []
    for core in range(NCORES):
        b, g = core // 2, core % 2
        xT = np.ascontiguousarray(x[b].T)  # [C, T]
        # wqk: cols [256p:256p+128] = q rows of pair p (.T), then k rows
        wqk = np.empty((C, 1024), np.float32)
        for p in range(NPAIR):
            r0 = 64 * (8 * g + 2 * p)
            wqk[:, 256 * p : 256 * p + 128] = W_qkv[r0 : r0 + 128, :].T
            wqk[:, 256 * p + 128 : 256 * p + 256] = W_qkv[
                1024 + r0 : 1024 + r0 + 128, :
            ].T
        wv = np.ascontiguousarray(W_qkv[2048 + CC * g : 2048 + CC * g + CC, :].T)
        # W_out columns for this core's output slice, rows in gathered order
        wout = np.ascontiguousarray(
            W_out[CC * g : CC * g + CC, :].T[perm, :]
        ).astype(ml_dtypes.bfloat16)
        # conv taps per channel-partition, lag-indexed; +1 residual on lag 0
        convw = np.zeros((128, NPAIR, K), np.float32)
        convb2 = np.zeros((128, NPAIR), np.float32)
        for p in range(NPAIR):
            base = CC * g + 128 * p
            for lag in range(K):
                w = conv_w[base : base + 128, K - 1 - lag]
                if lag == 0:
                    w = w + 1.0
                convw[:, p, lag] = w
            convb2[:, p] = conv_b[base : base + 128]
        in_maps.append(
            {
                "xT": xT.astype(qkv_np),
                "wqk": wqk.astype(qkv_np),
                "wv": wv.astype(qkv_np),
                "wout": wout,
                "masks": masks,
                "masksp": _make_masksp(),
                "ident": np.eye(128, dtype=ml_dtypes.bfloat16),
                "convw": convw,
                "convb": convb2,
            }
        )
    return in_maps


def assemble_output(results):
    out = np.empty((B, T, C), np.float32)
    for core in range(NCORES):
        b, g = core // 2, core % 2
        outT = np.asarray(results[core]["outT"], np.float32)  # [CC, T]
        out[b, :, CC * g : CC * g + CC] = outT.T
    return out


def kernel(x, W_qkv, W_out, conv_w, conv_b):
    if "nc" not in _NC_CACHE:
        _NC_CACHE["nc"] = build()
    nc = _NC_CACHE["nc"]
    in_maps = prepare_in_maps(x, W_qkv, W_out, conv_w, conv_b)
    res = run_bass_kernel_spmd(nc, in_maps, list(range(NCORES)))
    return assemble_output(res.results)
